# revision 34
# baseline (speedup 1.0000x reference)
import sys

import numpy as np

for p in ("/opt/trn_rl_repo",):
    if p not in sys.path:
        sys.path.insert(0, p)

import ml_dtypes  # noqa: E402

import concourse.tile as tile  # noqa: E402
from concourse import bacc, mybir  # noqa: E402
from concourse.bass_utils import run_bass_kernel_spmd  # noqa: E402

B, N, D = 128, 512, 512
NCORES = 8
BPC = B // NCORES  # 16 batch items per core
F32 = mybir.dt.float32
BF16 = mybir.dt.bfloat16
ACT_COPY = mybir.ActivationFunctionType.Copy


def _hadamard(n: int) -> np.ndarray:
    H = np.array([[1.0]], dtype=np.float32)
    base = np.array([[1.0, 1.0], [1.0, -1.0]], dtype=np.float32)
    while H.shape[0] < n:
        H = np.kron(H, base)
    return H


def _build():
    # y = H512 @ x @ H512 / 512 per item, via H512 = H2 (x) H256.
    # Per item: H2 input butterfly over the 256-row halves (GpSimd, bf16
    # out), 16 matmuls against H256 (K=256 as two accumulating K=128
    # passes) leaving t transposed in PSUM, a PSUM crossing (DVE) applying
    # the output-side H2 butterfly, 16 matmuls against H256/512, eviction
    # (Act), store.  Stage C runs one item behind stage B so the PE's
    # stage-B(b+1) overlaps the DVE crossing(b) instead of serializing.
    nc = bacc.Bacc("TRN2", target_bir_lowering=False, debug=False)
    x_d = nc.dram_tensor("x", [BPC, 4, 128, D], F32, kind="ExternalInput").ap()
    hc_d = nc.dram_tensor("hc", [128, 2, 2, 256], BF16, kind="ExternalInput").ap()
    y_d = nc.dram_tensor("y", [BPC, 4, 128, D], F32, kind="ExternalOutput").ap()

    with tile.TileContext(nc) as tc:
        with (
            tc.tile_pool(name="const", bufs=1) as cpool,
            tc.tile_pool(name="xp", bufs=12) as xpool,
            tc.tile_pool(name="xc", bufs=5) as xcpool,
            tc.tile_pool(name="tp", bufs=5) as ttpool,
            tc.tile_pool(name="tb", bufs=4) as tbpool,
            tc.tile_pool(name="yp", bufs=5) as ypool,
            tc.tile_pool(name="ps1", bufs=1, space="PSUM") as ps1pool,
            tc.tile_pool(name="ps2", bufs=1, space="PSUM") as ps2pool,
        ):
            hc = cpool.tile([128, 2, 2, 256], BF16)
            h256 = hc[:, 0]  # [128, 2, 256]: rows of H256, split in halves
            hs256 = hc[:, 1]  # H256 / 512

            def stage_in(b):
                xt = xpool.tile([128, 4, D], F32, tag="xt", name="xt")
                nc.sync.dma_start(xt[:], x_d[b].transpose([1, 0, 2]))
                # Input-side H2 butterfly: xcb[:, i, h] = xt[:, h] +/- xt[:, 2+h]
                xcb = xcpool.tile([128, 2, 2, D], BF16, tag="xcb", name="xcb")
                nc.gpsimd.tensor_add(xcb[:, 0], xt[:, 0:2], xt[:, 2:4])
                nc.gpsimd.tensor_sub(xcb[:, 1], xt[:, 0:2], xt[:, 2:4])
                return xcb

            def stage_b_cross(xcb):
                # Stage B: ps1[dt][d', i*256+q] = t[(i,q), dt*128+d']
                #        = sum_h xcb_i[:, h, dt-chunk].T @ H256[h-half]
                # ps1b (d-chunks 2,3) is filled first and evicted to SBUF by
                # Act, because the DVE crossing may read at most one PSUM
                # operand per instruction.
                ps1a = ps1pool.tile([128, 2, N], F32, tag="ps1a", name="ps1a")
                ps1b = ps1pool.tile([128, 2, N], F32, tag="ps1b", name="ps1b")
                for dt in (2, 3, 0, 1):
                    bank = ps1a if dt < 2 else ps1b
                    for i in range(2):
                        for h in range(2):
                            nc.tensor.matmul(
                                bank[:, dt % 2, i * 256 : (i + 1) * 256],
                                xcb[:, i, h, dt * 128 : (dt + 1) * 128],
                                h256[:, h],
                                start=(h == 0),
                                stop=(h == 1),
                            )
                tb = tbpool.tile([128, 2, N], BF16, tag="tb", name="tb")
                nc.vector.tensor_copy(tb[:], ps1b[:])
                # PSUM crossing (DVE): output-side H2 butterfly over d-halves.
                ttc = ttpool.tile([128, 4, N], BF16, tag="ttc", name="ttc")
                nc.vector.tensor_add(ttc[:, 0:2], ps1a[:], tb[:])
                nc.vector.tensor_sub(ttc[:, 2:4], ps1a[:], tb[:])
                return ttc

            def stage_c_out(b, ttc):
                # Stage C: ps2[nt][n'', ie*256+e']
                #   = sum_dt' ttc[:, 2*ie+dt', nt-chunk].T @ (H256/512)[dt'-half]
                yt = ypool.tile([128, 4, D], F32, tag="yt", name="yt")
                for half in range(2):
                    bank = ps2pool.tile(
                        [128, 2, D], F32, tag=f"ps2_{half}", name=f"ps2_{half}"
                    )
                    for nt2 in range(2):
                        nt = half * 2 + nt2
                        for ie in range(2):
                            for dtp in range(2):
                                nc.tensor.matmul(
                                    bank[:, nt2, ie * 256 : (ie + 1) * 256],
                                    ttc[:, 2 * ie + dtp, nt * 128 : (nt + 1) * 128],
                                    hs256[:, dtp],
                                    start=(dtp == 0),
                                    stop=(dtp == 1),
                                )
                    # Evict this half of y (Act) and store it (Act queue, so
                    # the store fires right after its eviction without blocking
                    # the SP queue's input DMAs).
                    nc.scalar.activation(
                        yt[:, half * 2 : half * 2 + 2], bank[:], ACT_COPY
                    )
                    nc.scalar.dma_start(
                        y_d[b, half * 2 : half * 2 + 2].transpose([1, 0, 2]),
                        yt[:, half * 2 : half * 2 + 2],
                    )

            # Software pipeline: stage C trails stage B by one item.
            prev = None  # (b, ttc)
            for b in range(BPC):
                xcb = stage_in(b)
                if b == 0:
                    # constants load after the first input DMA: stage B is the
                    # first consumer, the butterfly does not need them
                    nc.sync.dma_start(hc[:], hc_d[:])
                ttc = stage_b_cross(xcb)
                if prev is not None:
                    stage_c_out(*prev)
                prev = (b, ttc)
            stage_c_out(*prev)

    nc.compile()
    return nc


_NC = None


def kernel(x: np.ndarray) -> np.ndarray:
    global _NC
    if _NC is None:
        _NC = _build()
    x = np.ascontiguousarray(np.asarray(x), dtype=np.float32)
    H = _hadamard(256)
    # hc[p, 0, h, q] = H256[h*128+p, q]; hc[p, 1, h, q] = H256[h*128+p, q]/512
    hrows = H.reshape(2, 128, 256).transpose(1, 0, 2)  # [128, 2, 256]
    hc = np.stack([hrows, hrows / np.float32(512.0)], axis=1)  # [128, 2, 2, 256]
    hc = np.ascontiguousarray(hc.astype(ml_dtypes.bfloat16))
    xr = x.reshape(NCORES, BPC, 4, 128, D)
    in_maps = [{"x": xr[i], "hc": hc} for i in range(NCORES)]
    res = run_bass_kernel_spmd(_NC, in_maps, list(range(NCORES))).results
    return np.concatenate(
        [r["y"].reshape(BPC, N, D) for r in res], axis=0
    ).astype(np.float32)
